# revision 31
# baseline (speedup 1.0000x reference)
"""ClassAttention kernel for 8x TRN2 NeuronCores.

Reference computation (per batch element):
    qkv = x @ qkv_w.T + qkv_b                      # [N, 3C]
    q, k, v = split(qkv)                           # heads H=12, D=64
    s = softmax((q_cls . k) / sqrt(D))             # class-token query only
    cls = (s @ v) @ proj_w.T + proj_b              # [1, C]
    out = concat([cls, x[1:]])                     # rows 1..N pass through

Only the class token row changes, so the device computes just the [B, C]
cls output; rows 1..N are passed through on the host.

Sharding: data-parallel over batch, 8 batches per core, no collectives.

Algebraic structure (exploits the single class-token query; every device
matmul is arranged so the matmul OUTPUT free dim is tiny -- the wide
operand is always the stationary one, which the cost model/hardware
pipeline streams for free):
  - q (transposed, all batches, 64-row head blocks): qp2 = wq.T @ xcls
    with the q-bias folded in as a rank-1 ones-row matmul.  The k-bias
    cancels in softmax.
  - Wt[c, (b,h)] = wk_h.T @ q_bh per 64-row head block (wk pre-arranged
    host-side as [64, 12, C] so no partition offsets are needed); the
    scores then fold the whole k-projection into x-space:
    sT[n, h] = sum_c xT[c, n] Wt[c, bh].  No k vector is materialized.
  - softmax: e = exp(sT / 8) (scores are O(1): q.k of unit-variance
    inputs, so no max-shift; the 1/sqrt(D) lives in the exp scale);
    den = ones.T @ e (matmul); e_n = e * (64/den) via a ones-row
    broadcast matmul + DVE reciprocal.  The 64x keeps e_n inside
    fp8-e4m3 normal range; the 1/64 is removed at the oT evacuation.
  - the v-projection commutes with the attention average:
    ZT[c, h] = x_b.T @ p per batch, from an n-major fp8 copy of x.
    No v vector is ever materialized.
  - o (per head, all batches): oT[d, h, b] = wv_h.T @ ZT_b into 64-row
    psum blocks -- no diagonal extraction; proj (transposed):
    clsT[j, b] = sum_h wp64_h.T @ oT[:, h, :] with K=64 chunks.  v-bias
    and proj bias fold into a host-side add: pb_eff = proj_b +
    vb @ proj_w.T (weight-only algebra).

Both x layouts (c-major for scores, n-major for the average) and all
weights stream as fp8-e4m3: ~9.5 MB per core, the modeled DMA floor.
Measured full-output rel err 6.2e-3 on hardware vs the 2e-2 gate; psum
accumulation is fp32 and e/den stay bf16/f32.

Per-slot pipeline (slot k): Z(k-2) | V(k-2) | A(k) scores | exp(k) |
den/bcast/norm(k).  The softmax chain has ~1.5 slots of slack before
Z(k) consumes e_n(k), so slots pace at the DMA rate (~2.46us/batch).
The DMA stream is issued in consumption order; wp64 is issued after
the full x stream (its bytes would otherwise delay the last batch's
x2 tile, which gates the endgame), and the endgame runs
Z | O(4..8) | proj in one staged chain.
"""

import functools

import numpy as np
import ml_dtypes

import concourse.bass as bass
import concourse.tile as tile
from concourse import bacc, mybir, masks
from concourse import bass_utils

BF16 = mybir.dt.bfloat16
F8 = mybir.dt.float8e4
F32 = mybir.dt.float32
NPBF16 = ml_dtypes.bfloat16
NPF8 = ml_dtypes.float8_e4m3

B, N, C = 64, 577, 768
H, D = 12, 64
NCORES = 8
BPC = B // NCORES          # 8 batches per core
CT = C // 128              # 6 chunks of the feature dim
SCALE = D ** -0.5          # applied at the exp stage

# token tiles: 5 chunks of <=128 (n on partitions); tiles in TP_GROUPS are
# PE-transposed on chip, tiles in DMA_TILES are DMA'd from the x2 layout
T_TILES = [(0, 128), (128, 128), (256, 128), (384, 128), (512, 65)]
NTT = len(T_TILES)

# scheduling knobs (overridable for sweeps)
CFG = {
    "skew": 2,         # Z(k-skew) emitted at top of slot k
    "o_slot": 6,       # slot emitting O for batches 0-3 (needs V(3): >= skew+3)
    "sc_bufs": 2,
}


def build_module():
    nc = bacc.Bacc("TRN2", target_bir_lowering=False, debug=False)

    xT_d = nc.dram_tensor("xT", [C, BPC, N], F8, kind="ExternalInput")
    x2_d = nc.dram_tensor("x2", [BPC, N, C], F8, kind="ExternalInput")
    wq_d = nc.dram_tensor("wq", [C, C], F8, kind="ExternalInput")      # [c, o]
    wk64_d = nc.dram_tensor("wk64", [D, H, C], F8, kind="ExternalInput")
    wv_d = nc.dram_tensor("wv", [C, C], F8, kind="ExternalInput")      # [c, o]
    wp64_d = nc.dram_tensor("wp64", [D, H, C], F8, kind="ExternalInput")
    xcls_d = nc.dram_tensor("xcls", [C, BPC], F8, kind="ExternalInput")
    qbr_d = nc.dram_tensor("qbr", [1, C], BF16, kind="ExternalInput")
    clsT_d = nc.dram_tensor("clsT", [CT, 128, BPC], F32, kind="ExternalOutput")

    AF = mybir.ActivationFunctionType

    with tile.TileContext(nc) as tc:
        with (
            tc.tile_pool(name="const", bufs=1) as const,
            tc.tile_pool(name="xp", bufs=5) as xp,
            tc.tile_pool(name="x2p", bufs=6) as x2p,
            tc.tile_pool(name="sm", bufs=3) as sm,
            tc.tile_pool(name="ps", bufs=2, space="PSUM") as ps,
        ):
            xbs, x2ts, st = {}, {}, {}

            def load_xb(b):
                xb = xp.tile([128, CT, N], F8, tag="xb")
                nc.sync.dma_start(
                    xb[:],
                    xT_d.ap()[:, b, :].rearrange("(a p) t -> p a t", p=128))
                xbs[b] = xb

            def load_x2(b):
                x2t = x2p.tile([128, NTT, C], F8, tag="x2t")
                nc.sync.dma_start(
                    x2t[:, 0:4, :],
                    x2_d.ap()[b, 0:512, :].rearrange("(a p) c -> p a c", p=128))
                to, tw = T_TILES[4]
                nc.sync.dma_start(
                    x2t[:tw, 4, :], x2_d.ap()[b, to:to + tw, :])
                x2ts[b] = x2t

            # ---- DMAs, in the order the pipeline consumes them ----
            load_xb(0)
            xcls = const.tile([128, CT, BPC], F8, tag="xcls")
            nc.sync.dma_start(
                xcls[:], xcls_d.ap().rearrange("(a p) b -> p a b", p=128))
            qbr = const.tile([1, CT, 128], BF16, tag="qbr")
            nc.sync.dma_start(qbr[:], qbr_d.ap())
            wq = const.tile([128, CT, C], F8, tag="wq")
            nc.sync.dma_start(
                wq[:], wq_d.ap().rearrange("(a p) o -> p a o", p=128))
            wk64 = const.tile([D, H, C], F8, tag="wk64")
            nc.sync.dma_start(wk64[:], wk64_d.ap())
            wv = const.tile([128, CT, C], F8, tag="wv")
            wp64 = const.tile([D, H, C], F8, tag="wp64")
            load_xb(1)
            load_xb(2)
            load_x2(0)
            load_xb(3)
            load_x2(1)
            nc.sync.dma_start(
                wv[:], wv_d.ap().rearrange("(a p) o -> p a o", p=128))

            # ---- constants ----
            ones_n = const.tile([128, 1], BF16, tag="ones_n")
            nc.vector.memset(ones_n[:], 1.0)
            ones1 = const.tile([1, 128], BF16, tag="ones1")
            nc.vector.memset(ones1[:], 1.0)

            Wt = const.tile([128, CT, BPC * H], F8, tag="Wt")
            ZT_all = const.tile([128, CT, BPC, H], F8, tag="ZT_all")

            # ---- q for all 8 batches, 64-row blocks: qp2[d, j, ci, b] ----
            # (j = o-half within a 128-chunk; head h = 2*ci + j)
            qp2 = ps.tile([D, 2, CT, BPC], F32, tag="sc", bufs=CFG["sc_bufs"])
            for ci in range(CT):
                for j in range(2):
                    o0 = ci * 128 + 64 * j
                    for cj in range(CT):
                        nc.tensor.matmul(
                            qp2[:, j, ci, :],
                            wq[:, cj, o0:o0 + 64],
                            xcls[:, cj, :],
                            start=(cj == 0), stop=False)
                    # rank-1 ones-row term adds the q bias: qb[o] x ones[b]
                    nc.tensor.matmul(
                        qp2[:, j, ci, :], qbr[:, ci, 64 * j:64 * (j + 1)],
                        ones1[:, 0:BPC],
                        start=False, stop=True)
            qT8 = const.tile([D, 2, CT, BPC], F8, tag="qT8")
            nc.vector.tensor_copy(qT8[:], qp2[:])

            # ---- Wt[c, cj, b*12+h] = wk_h.T @ q_bh (64-row head blocks) ----
            WtV = Wt[:].rearrange("p a (b h) -> p a b h", h=H)
            for half in range(2):
                c0 = CT // 2 * half
                wtp = ps.tile([128, CT // 2, BPC * H], F32, tag="wt", bufs=2)
                wtpV = wtp[:].rearrange("p a (b h) -> p a b h", h=H)
                for cj in range(c0, c0 + CT // 2):
                    for h in range(H):
                        j, oj = h % 2, h // 2
                        nc.tensor.matmul(
                            wtpV[:, cj - c0, :, h],
                            wk64[:, h, cj * 128:(cj + 1) * 128],
                            qT8[:, j, oj, :],
                            start=True, stop=True)
                nc.vector.tensor_copy(
                    WtV[:, c0:c0 + CT // 2, :, :], wtpV[:, :, :, :])

            # ---- per-batch software pipeline ----
            def emit_A(b):
                # transposed scores: sc[n, ti, h]
                xb = xbs[b]
                sc = ps.tile([128, NTT, H], F32, tag="sc",
                             bufs=CFG["sc_bufs"])
                for ti, (to, tw) in enumerate(T_TILES):
                    for ci in range(CT):
                        nc.tensor.matmul(
                            sc[:tw, ti, :],
                            xb[:, ci, to:to + tw],
                            Wt[:, ci, b * H:(b + 1) * H],
                            start=(ci == 0), stop=(ci == CT - 1))
                st[b] = {"sc": sc}

            def emit_exp(b):
                s = st[b]
                e = sm.tile([128, NTT, H], BF16, tag="e")
                nc.scalar.activation(
                    e[:, 0:4, :].rearrange("p a h -> p (a h)"),
                    s["sc"][:, 0:4, :].rearrange("p a h -> p (a h)"),
                    AF.Exp, bias=0.0, scale=SCALE)
                tw = T_TILES[-1][1]
                nc.scalar.activation(
                    e[:tw, 4, :], s["sc"][:tw, 4, :],
                    AF.Exp, bias=0.0, scale=SCALE)
                s["e"] = e

            def emit_den(b):
                # den_row[1, h] = sum_n e[n, h], then /64 for fp8 range
                s = st[b]
                e = s["e"]
                dn = ps.tile([1, H], F32, tag="dn", bufs=1)
                for ti, (to, tw) in enumerate(T_TILES):
                    nc.tensor.matmul(
                        dn[:, :], ones_n[:tw, :], e[:tw, ti, :],
                        start=(ti == 0), stop=(ti == NTT - 1))
                dsb = sm.tile([1, H], BF16, tag="dsb")
                nc.vector.tensor_scalar_mul(dsb[:], dn[:], 1.0 / 64.0)
                s["dsb"] = dsb

            def emit_bcast(b):
                s = st[b]
                rb = ps.tile([128, H], F32, tag="dn", bufs=1)
                nc.tensor.matmul(
                    rb[:, :], ones1[:, :], s["dsb"][:, :],
                    start=True, stop=True)
                s["rb"] = rb

            def emit_norm(b):
                # e_n = e * (64/den)  (broadcast over the token tiles)
                s = st[b]
                rsb = sm.tile([128, H], F32, tag="rsb")
                nc.vector.reciprocal(rsb[:], s["rb"][:])
                e = s["e"]
                e_n = sm.tile([128, NTT, H], F8, tag="e_n")
                rext = bass.AP(
                    rsb.tensor, rsb.offset,
                    [rsb.ap[0], [0, 4], rsb.ap[1]])
                nc.vector.tensor_tensor(
                    e_n[:, 0:4, :], e[:, 0:4, :], rext, mybir.AluOpType.mult)
                tw = T_TILES[-1][1]
                nc.vector.tensor_tensor(
                    e_n[:tw, 4, :], e[:tw, 4, :], rsb[:tw, :],
                    mybir.AluOpType.mult)
                s["e_n"] = e_n

            def emit_Z(b):
                # ZT[c, cj, h] = x_b.T @ p  (attention average, transposed)
                s = st.pop(b)
                del xbs[b]
                x2t = x2ts.pop(b)
                e_n = s["e_n"]
                zt = ps.tile([128, CT, H], F32, tag="zt", bufs=1)
                for cj in range(CT):
                    for ti, (to, tw) in enumerate(T_TILES):
                        nc.tensor.matmul(
                            zt[:, cj, :],
                            x2t[:tw, ti, cj * 128:(cj + 1) * 128],
                            e_n[:tw, ti, :],
                            start=(ti == 0), stop=(ti == NTT - 1))
                nc.vector.tensor_copy(ZT_all[:, :, b, :], zt[:, :, :])

            # o per head: po[d, h, b]; proj: clsT[j, cj, b]; both staged
            # by batch group so most of it runs inside the pipeline
            oT = const.tile([D, H, BPC], F8, tag="oT")
            cls_sb = const.tile([128, CT, BPC], F32, tag="cls_sb")

            def emit_O(b0, b1):
                po = ps.tile([D, H, b1 - b0], F32, tag="sc",
                             bufs=CFG["sc_bufs"])
                for h in range(H):
                    for ci in range(CT):
                        nc.tensor.matmul(
                            po[:, h, :],
                            wv[:, ci, D * h:D * (h + 1)],
                            ZT_all[:, ci, b0:b1, h],
                            start=(ci == 0), stop=(ci == CT - 1))
                nc.vector.tensor_scalar_mul(
                    oT[:, :, b0:b1], po[:, :, :], 1.0 / 64.0)

            def emit_proj(b0, b1):
                ct = ps.tile([128, CT, b1 - b0], F32, tag="sc",
                             bufs=CFG["sc_bufs"])
                for cj in range(CT):
                    for h in range(H):
                        nc.tensor.matmul(
                            ct[:, cj, :],
                            wp64[:, h, cj * 128:(cj + 1) * 128],
                            oT[:, h, b0:b1],
                            start=(h == 0), stop=(h == H - 1))
                nc.vector.tensor_copy(cls_sb[:, :, b0:b1], ct[:, :, :])
                nc.sync.dma_start(
                    clsT_d.ap().rearrange("a p b -> p a b")[:, :, b0:b1],
                    cls_sb[:, :, b0:b1])

            for k in range(BPC):
                if k + 4 < BPC:
                    load_xb(k + 4)
                if k + 2 < BPC:
                    load_x2(k + 2)
                if k >= CFG["skew"]:
                    emit_Z(k - CFG["skew"])
                if k == CFG["o_slot"]:
                    emit_O(0, 4)
                emit_A(k)
                emit_exp(k)
                emit_den(k)
                emit_bcast(k)
                emit_norm(k)
            nc.sync.dma_start(wp64[:], wp64_d.ap())
            for b in range(BPC - CFG["skew"], BPC):
                emit_Z(b)
            emit_O(4, BPC)
            emit_proj(0, BPC)

    nc.compile()
    return nc


@functools.lru_cache(maxsize=1)
def _module():
    return build_module()


def make_in_maps(x, qkv_w, qkv_b, proj_w, proj_b):
    x = np.asarray(x, dtype=np.float32)
    qkv_w = np.asarray(qkv_w, dtype=np.float32)
    qkv_b = np.asarray(qkv_b, dtype=np.float32)
    proj_w = np.asarray(proj_w, dtype=np.float32)
    proj_b = np.asarray(proj_b, dtype=np.float32)

    wq = np.ascontiguousarray(qkv_w[:C].T).astype(NPF8)             # [c, o]
    # wk64[d, h, c] = qkv_w[C + 64h + d, c]
    wk64 = np.ascontiguousarray(
        qkv_w[C:2 * C].reshape(H, D, C).transpose(1, 0, 2)).astype(NPF8)
    wv = np.ascontiguousarray(qkv_w[2 * C:].T).astype(NPF8)         # [c, o]
    # wp64[d, h, j] = proj_w[j, 64h + d]
    wp64 = np.ascontiguousarray(
        proj_w.T.reshape(H, D, C).transpose(1, 0, 2)).astype(NPF8)
    # q bias as a [1, C] row (scores are scaled by 1/8 at the exp stage)
    qbr = qkv_b[:C].astype(NPBF16).reshape(1, C)

    in_maps = []
    for i in range(NCORES):
        xs = x[i * BPC:(i + 1) * BPC]                               # [8, N, C]
        x8 = xs.astype(NPF8)
        xT = np.ascontiguousarray(x8.transpose(2, 0, 1))            # [C, 8, N]
        xcls = np.ascontiguousarray(x8[:, 0, :].T)                  # [C, 8]
        in_maps.append({
            "xT": xT, "x2": x8, "wq": wq, "wk64": wk64, "wv": wv,
            "wp64": wp64, "xcls": xcls, "qbr": qbr,
        })
    return in_maps


def kernel(x, qkv_w, qkv_b, proj_w, proj_b):
    nc = _module()
    in_maps = make_in_maps(x, qkv_w, qkv_b, proj_w, proj_b)
    res = bass_utils.run_bass_kernel_spmd(
        nc, in_maps, core_ids=list(range(NCORES)))
    # v bias contributes exactly (vb @ proj_w.T); fold into the proj bias
    # and add on the host (weight-only algebra).
    qkv_b = np.asarray(qkv_b, dtype=np.float32)
    pb_eff = np.asarray(proj_b, dtype=np.float32) + qkv_b[2 * C:] @ np.asarray(
        proj_w, dtype=np.float32).T
    out = np.array(np.asarray(x), dtype=np.float32, copy=True)
    for i in range(NCORES):
        clsT = np.asarray(res.results[i]["clsT"])                   # [6,128,8]
        cls = clsT.reshape(C, BPC).T + pb_eff                       # [8, C]
        out[i * BPC:(i + 1) * BPC, 0, :] = cls
    return out


# revision 32
# speedup vs baseline: 1.0053x; 1.0053x over previous
"""ClassAttention kernel for 8x TRN2 NeuronCores.

Reference computation (per batch element):
    qkv = x @ qkv_w.T + qkv_b                      # [N, 3C]
    q, k, v = split(qkv)                           # heads H=12, D=64
    s = softmax((q_cls . k) / sqrt(D))             # class-token query only
    cls = (s @ v) @ proj_w.T + proj_b              # [1, C]
    out = concat([cls, x[1:]])                     # rows 1..N pass through

Only the class token row changes, so the device computes just the [B, C]
cls output; rows 1..N are passed through on the host.

Sharding: data-parallel over batch, 8 batches per core, no collectives.

Algebraic structure (exploits the single class-token query; every device
matmul is arranged so the matmul OUTPUT free dim is tiny -- the wide
operand is always the stationary one, which the cost model/hardware
pipeline streams for free):
  - q (transposed, all batches, 64-row head blocks): qp2 = wq.T @ xcls
    with the q-bias folded in as a rank-1 ones-row matmul.  The k-bias
    cancels in softmax.
  - Wt[c, (b,h)] = wk_h.T @ q_bh per 64-row head block (wk pre-arranged
    host-side as [64, 12, C] so no partition offsets are needed); the
    scores then fold the whole k-projection into x-space:
    sT[n, h] = sum_c xT[c, n] Wt[c, bh].  No k vector is materialized.
  - softmax: e = exp(sT / 8) (scores are O(1): q.k of unit-variance
    inputs, so no max-shift; the 1/sqrt(D) lives in the exp scale);
    den = ones.T @ e (matmul); e_n = e * (64/den) via a ones-row
    broadcast matmul + DVE reciprocal.  The 64x keeps e_n inside
    fp8-e4m3 normal range; the 1/64 is removed at the oT evacuation.
  - the v-projection commutes with the attention average:
    ZT[c, h] = x_b.T @ p per batch, from an n-major fp8 copy of x.
    No v vector is ever materialized.
  - o (per head, all batches): oT[d, h, b] = wv_h.T @ ZT_b into 64-row
    psum blocks -- no diagonal extraction; proj (transposed):
    clsT[j, b] = sum_h wp64_h.T @ oT[:, h, :] with K=64 chunks.  v-bias
    and proj bias fold into a host-side add: pb_eff = proj_b +
    vb @ proj_w.T (weight-only algebra).

Both x layouts (c-major for scores, n-major for the average) and all
weights stream as fp8-e4m3: ~9.5 MB per core, the modeled DMA floor.
Measured full-output rel err 6.2e-3 on hardware vs the 2e-2 gate; psum
accumulation is fp32 and e/den stay bf16/f32.

Per-slot pipeline (slot k): Z(k-2) | V(k-2) | A(k) scores | exp(k) |
den/bcast/norm(k).  The softmax chain has ~1.5 slots of slack before
Z(k) consumes e_n(k), so slots pace at the DMA rate (~2.46us/batch).
The DMA stream is issued in consumption order; wp64 is issued after
the full x stream (its bytes would otherwise delay the last batch's
x2 tile, which gates the endgame), and the endgame runs
Z | O(4..8) | proj in one staged chain.
"""

import functools

import numpy as np
import ml_dtypes

import concourse.bass as bass
import concourse.tile as tile
from concourse import bacc, mybir, masks
from concourse import bass_utils

BF16 = mybir.dt.bfloat16
F8 = mybir.dt.float8e4
F32 = mybir.dt.float32
NPBF16 = ml_dtypes.bfloat16
NPF8 = ml_dtypes.float8_e4m3

B, N, C = 64, 577, 768
H, D = 12, 64
NCORES = 8
BPC = B // NCORES          # 8 batches per core
CT = C // 128              # 6 chunks of the feature dim
SCALE = D ** -0.5          # applied at the exp stage

# token tiles: 5 chunks of <=128 (n on partitions); tiles in TP_GROUPS are
# PE-transposed on chip, tiles in DMA_TILES are DMA'd from the x2 layout
T_TILES = [(0, 128), (128, 128), (256, 128), (384, 128), (512, 65)]
NTT = len(T_TILES)

# scheduling knobs (overridable for sweeps)
CFG = {
    "skew": 2,         # Z(k-skew) emitted at top of slot k
    "o_slot": 6,       # slot emitting O for batches 0-3 (needs V(3): >= skew+3)
    "sc_bufs": 2,
}


def build_module():
    nc = bacc.Bacc("TRN2", target_bir_lowering=False, debug=False)

    xT_d = nc.dram_tensor("xT", [C, BPC, N], F8, kind="ExternalInput")
    x2_d = nc.dram_tensor("x2", [BPC, N, C], F8, kind="ExternalInput")
    wq_d = nc.dram_tensor("wq", [C, C], F8, kind="ExternalInput")      # [c, o]
    wk64_d = nc.dram_tensor("wk64", [D, H, C], F8, kind="ExternalInput")
    wv_d = nc.dram_tensor("wv", [C, C], F8, kind="ExternalInput")      # [c, o]
    wp64_d = nc.dram_tensor("wp64", [D, H, C], F8, kind="ExternalInput")
    xcls_d = nc.dram_tensor("xcls", [C, BPC], F8, kind="ExternalInput")
    qbr_d = nc.dram_tensor("qbr", [1, C], BF16, kind="ExternalInput")
    clsT_d = nc.dram_tensor("clsT", [CT, 128, BPC], F32, kind="ExternalOutput")

    AF = mybir.ActivationFunctionType

    with tile.TileContext(nc) as tc:
        with (
            tc.tile_pool(name="const", bufs=1) as const,
            tc.tile_pool(name="xp", bufs=5) as xp,
            tc.tile_pool(name="x2p", bufs=6) as x2p,
            tc.tile_pool(name="sm", bufs=3) as sm,
            tc.tile_pool(name="ps", bufs=2, space="PSUM") as ps,
        ):
            xbs, x2ts, st = {}, {}, {}

            def load_xb(b):
                xb = xp.tile([128, CT, N], F8, tag="xb")
                nc.sync.dma_start(
                    xb[:],
                    xT_d.ap()[:, b, :].rearrange("(a p) t -> p a t", p=128))
                xbs[b] = xb

            def load_x2(b):
                x2t = x2p.tile([128, NTT, C], F8, tag="x2t")
                nc.sync.dma_start(
                    x2t[:, 0:4, :],
                    x2_d.ap()[b, 0:512, :].rearrange("(a p) c -> p a c", p=128))
                to, tw = T_TILES[4]
                nc.sync.dma_start(
                    x2t[:tw, 4, :], x2_d.ap()[b, to:to + tw, :])
                x2ts[b] = x2t

            # ---- DMAs, in the order the pipeline consumes them ----
            load_xb(0)
            xcls = const.tile([128, CT, BPC], F8, tag="xcls")
            nc.sync.dma_start(
                xcls[:], xcls_d.ap().rearrange("(a p) b -> p a b", p=128))
            qbr = const.tile([1, CT, 128], BF16, tag="qbr")
            nc.sync.dma_start(qbr[:], qbr_d.ap())
            wq = const.tile([128, CT, C], F8, tag="wq")
            nc.sync.dma_start(
                wq[:], wq_d.ap().rearrange("(a p) o -> p a o", p=128))
            wk64 = const.tile([D, H, C], F8, tag="wk64")
            nc.sync.dma_start(wk64[:], wk64_d.ap())
            wv = const.tile([128, CT, C], F8, tag="wv")
            wp64 = const.tile([D, H, C], F8, tag="wp64")
            load_xb(1)
            load_xb(2)
            load_x2(0)
            load_xb(3)
            load_x2(1)

            # ---- constants ----
            ones_n = const.tile([128, 1], BF16, tag="ones_n")
            nc.vector.memset(ones_n[:], 1.0)
            ones1 = const.tile([1, 128], BF16, tag="ones1")
            nc.vector.memset(ones1[:], 1.0)

            Wt = const.tile([128, CT, BPC * H], F8, tag="Wt")
            ZT_all = const.tile([128, CT, BPC, H], F8, tag="ZT_all")

            # ---- q for all 8 batches, 64-row blocks: qp2[d, j, ci, b] ----
            # (j = o-half within a 128-chunk; head h = 2*ci + j)
            qp2 = ps.tile([D, 2, CT, BPC], F32, tag="sc", bufs=CFG["sc_bufs"])
            for ci in range(CT):
                for j in range(2):
                    o0 = ci * 128 + 64 * j
                    for cj in range(CT):
                        nc.tensor.matmul(
                            qp2[:, j, ci, :],
                            wq[:, cj, o0:o0 + 64],
                            xcls[:, cj, :],
                            start=(cj == 0), stop=False)
                    # rank-1 ones-row term adds the q bias: qb[o] x ones[b]
                    nc.tensor.matmul(
                        qp2[:, j, ci, :], qbr[:, ci, 64 * j:64 * (j + 1)],
                        ones1[:, 0:BPC],
                        start=False, stop=True)
            qT8 = const.tile([D, 2, CT, BPC], F8, tag="qT8")
            nc.vector.tensor_copy(qT8[:], qp2[:])

            # ---- Wt[c, cj, b*12+h] = wk_h.T @ q_bh (64-row head blocks) ----
            WtV = Wt[:].rearrange("p a (b h) -> p a b h", h=H)
            for half in range(2):
                c0 = CT // 2 * half
                wtp = ps.tile([128, CT // 2, BPC * H], F32, tag="wt", bufs=2)
                wtpV = wtp[:].rearrange("p a (b h) -> p a b h", h=H)
                for cj in range(c0, c0 + CT // 2):
                    for h in range(H):
                        j, oj = h % 2, h // 2
                        nc.tensor.matmul(
                            wtpV[:, cj - c0, :, h],
                            wk64[:, h, cj * 128:(cj + 1) * 128],
                            qT8[:, j, oj, :],
                            start=True, stop=True)
                nc.vector.tensor_copy(
                    WtV[:, c0:c0 + CT // 2, :, :], wtpV[:, :, :, :])

            # ---- per-batch software pipeline ----
            def emit_A(b):
                # transposed scores: sc[n, ti, h]
                xb = xbs[b]
                sc = ps.tile([128, NTT, H], F32, tag="sc",
                             bufs=CFG["sc_bufs"])
                for ti, (to, tw) in enumerate(T_TILES):
                    for ci in range(CT):
                        nc.tensor.matmul(
                            sc[:tw, ti, :],
                            xb[:, ci, to:to + tw],
                            Wt[:, ci, b * H:(b + 1) * H],
                            start=(ci == 0), stop=(ci == CT - 1))
                st[b] = {"sc": sc}

            def emit_exp(b):
                s = st[b]
                e = sm.tile([128, NTT, H], BF16, tag="e")
                nc.scalar.activation(
                    e[:, 0:4, :].rearrange("p a h -> p (a h)"),
                    s["sc"][:, 0:4, :].rearrange("p a h -> p (a h)"),
                    AF.Exp, bias=0.0, scale=SCALE)
                tw = T_TILES[-1][1]
                nc.scalar.activation(
                    e[:tw, 4, :], s["sc"][:tw, 4, :],
                    AF.Exp, bias=0.0, scale=SCALE)
                s["e"] = e

            def emit_den(b):
                # den_row[1, h] = sum_n e[n, h], then /64 for fp8 range
                s = st[b]
                e = s["e"]
                dn = ps.tile([1, H], F32, tag="dn", bufs=1)
                for ti, (to, tw) in enumerate(T_TILES):
                    nc.tensor.matmul(
                        dn[:, :], ones_n[:tw, :], e[:tw, ti, :],
                        start=(ti == 0), stop=(ti == NTT - 1))
                dsb = sm.tile([1, H], BF16, tag="dsb")
                nc.vector.tensor_scalar_mul(dsb[:], dn[:], 1.0 / 64.0)
                s["dsb"] = dsb

            def emit_bcast(b):
                s = st[b]
                rb = ps.tile([128, H], F32, tag="dn", bufs=1)
                nc.tensor.matmul(
                    rb[:, :], ones1[:, :], s["dsb"][:, :],
                    start=True, stop=True)
                s["rb"] = rb

            def emit_norm(b):
                # e_n = e * (64/den)  (broadcast over the token tiles)
                s = st[b]
                rsb = sm.tile([128, H], F32, tag="rsb")
                nc.vector.reciprocal(rsb[:], s["rb"][:])
                e = s["e"]
                e_n = sm.tile([128, NTT, H], F8, tag="e_n")
                rext = bass.AP(
                    rsb.tensor, rsb.offset,
                    [rsb.ap[0], [0, 4], rsb.ap[1]])
                nc.vector.tensor_tensor(
                    e_n[:, 0:4, :], e[:, 0:4, :], rext, mybir.AluOpType.mult)
                tw = T_TILES[-1][1]
                nc.vector.tensor_tensor(
                    e_n[:tw, 4, :], e[:tw, 4, :], rsb[:tw, :],
                    mybir.AluOpType.mult)
                s["e_n"] = e_n

            def emit_Z(b):
                # ZT[c, cj, h] = x_b.T @ p  (attention average, transposed)
                s = st.pop(b)
                del xbs[b]
                x2t = x2ts.pop(b)
                e_n = s["e_n"]
                zt = ps.tile([128, CT, H], F32, tag="zt", bufs=1)
                for cj in range(CT):
                    for ti, (to, tw) in enumerate(T_TILES):
                        nc.tensor.matmul(
                            zt[:, cj, :],
                            x2t[:tw, ti, cj * 128:(cj + 1) * 128],
                            e_n[:tw, ti, :],
                            start=(ti == 0), stop=(ti == NTT - 1))
                nc.vector.tensor_copy(ZT_all[:, :, b, :], zt[:, :, :])

            # o per head: po[d, h, b]; proj: clsT[j, cj, b]; both staged
            # by batch group so most of it runs inside the pipeline
            oT = const.tile([D, H, BPC], F8, tag="oT")
            cls_sb = const.tile([128, CT, BPC], F32, tag="cls_sb")

            def emit_O(b0, b1):
                po = ps.tile([D, H, b1 - b0], F32, tag="sc",
                             bufs=CFG["sc_bufs"])
                for h in range(H):
                    for ci in range(CT):
                        nc.tensor.matmul(
                            po[:, h, :],
                            wv[:, ci, D * h:D * (h + 1)],
                            ZT_all[:, ci, b0:b1, h],
                            start=(ci == 0), stop=(ci == CT - 1))
                nc.vector.tensor_scalar_mul(
                    oT[:, :, b0:b1], po[:, :, :], 1.0 / 64.0)

            def emit_proj(b0, b1):
                ct = ps.tile([128, CT, b1 - b0], F32, tag="sc",
                             bufs=CFG["sc_bufs"])
                for cj in range(CT):
                    for h in range(H):
                        nc.tensor.matmul(
                            ct[:, cj, :],
                            wp64[:, h, cj * 128:(cj + 1) * 128],
                            oT[:, h, b0:b1],
                            start=(h == 0), stop=(h == H - 1))
                nc.vector.tensor_copy(cls_sb[:, :, b0:b1], ct[:, :, :])
                nc.sync.dma_start(
                    clsT_d.ap().rearrange("a p b -> p a b")[:, :, b0:b1],
                    cls_sb[:, :, b0:b1])

            for k in range(BPC):
                if k + 4 < BPC:
                    load_xb(k + 4)
                if k + 2 < BPC:
                    load_x2(k + 2)
                if k >= CFG["skew"]:
                    emit_Z(k - CFG["skew"])
                emit_A(k)
                emit_exp(k)
                emit_den(k)
                emit_bcast(k)
                emit_norm(k)
            nc.sync.dma_start(
                wv[:], wv_d.ap().rearrange("(a p) o -> p a o", p=128))
            nc.sync.dma_start(wp64[:], wp64_d.ap())
            for b in range(BPC - CFG["skew"], BPC):
                emit_Z(b)
            emit_O(0, BPC)
            emit_proj(0, BPC)

    nc.compile()
    return nc


@functools.lru_cache(maxsize=1)
def _module():
    return build_module()


def make_in_maps(x, qkv_w, qkv_b, proj_w, proj_b):
    x = np.asarray(x, dtype=np.float32)
    qkv_w = np.asarray(qkv_w, dtype=np.float32)
    qkv_b = np.asarray(qkv_b, dtype=np.float32)
    proj_w = np.asarray(proj_w, dtype=np.float32)
    proj_b = np.asarray(proj_b, dtype=np.float32)

    wq = np.ascontiguousarray(qkv_w[:C].T).astype(NPF8)             # [c, o]
    # wk64[d, h, c] = qkv_w[C + 64h + d, c]
    wk64 = np.ascontiguousarray(
        qkv_w[C:2 * C].reshape(H, D, C).transpose(1, 0, 2)).astype(NPF8)
    wv = np.ascontiguousarray(qkv_w[2 * C:].T).astype(NPF8)         # [c, o]
    # wp64[d, h, j] = proj_w[j, 64h + d]
    wp64 = np.ascontiguousarray(
        proj_w.T.reshape(H, D, C).transpose(1, 0, 2)).astype(NPF8)
    # q bias as a [1, C] row (scores are scaled by 1/8 at the exp stage)
    qbr = qkv_b[:C].astype(NPBF16).reshape(1, C)

    in_maps = []
    for i in range(NCORES):
        xs = x[i * BPC:(i + 1) * BPC]                               # [8, N, C]
        x8 = xs.astype(NPF8)
        xT = np.ascontiguousarray(x8.transpose(2, 0, 1))            # [C, 8, N]
        xcls = np.ascontiguousarray(x8[:, 0, :].T)                  # [C, 8]
        in_maps.append({
            "xT": xT, "x2": x8, "wq": wq, "wk64": wk64, "wv": wv,
            "wp64": wp64, "xcls": xcls, "qbr": qbr,
        })
    return in_maps


def kernel(x, qkv_w, qkv_b, proj_w, proj_b):
    nc = _module()
    in_maps = make_in_maps(x, qkv_w, qkv_b, proj_w, proj_b)
    res = bass_utils.run_bass_kernel_spmd(
        nc, in_maps, core_ids=list(range(NCORES)))
    # v bias contributes exactly (vb @ proj_w.T); fold into the proj bias
    # and add on the host (weight-only algebra).
    qkv_b = np.asarray(qkv_b, dtype=np.float32)
    pb_eff = np.asarray(proj_b, dtype=np.float32) + qkv_b[2 * C:] @ np.asarray(
        proj_w, dtype=np.float32).T
    out = np.array(np.asarray(x), dtype=np.float32, copy=True)
    for i in range(NCORES):
        clsT = np.asarray(res.results[i]["clsT"])                   # [6,128,8]
        cls = clsT.reshape(C, BPC).T + pb_eff                       # [8, C]
        out[i * BPC:(i + 1) * BPC, 0, :] = cls
    return out
